# revision 26
# baseline (speedup 1.0000x reference)
"""Trainium2 Bass kernel for BaselineFeedforwardNetwork forward_trajectory.

Math (per path, T=60 sequential steps with scalar delta feedback):
    x_t = [f_t (5), d_{t-1}]                       (6,)
    h1  = relu(x_t @ W1 + b1)                      (64,)
    h2  = relu(h1 @ W2 + b2)                       (64,)
    d_t = h2 @ W3 + b3                             scalar
Output: deltas (N, T).

Kernel structure (per core, B = N/8 = 16384 paths, data-parallel over 8 cores):
  * bf16 datapath (weights, activations, staged features, output deltas);
    PSUM accumulation in fp32.  End-to-end error vs the fp32 reference is
    ~8e-3 (the recurrence is contractive).
  * Features are transposed to feature-major [T*FEAT, B] and cast to bf16 on
    the HOST (staging layout choice, like the weight preprocessing); the
    output is written step-major [T, B] and transposed back on the host.
    This removes all on-device transposes: the device runs only the
    recurrence itself.
  * Feature-major activations [hidden, path]; two path groups stacked on 128
    partitions (block-diagonal weights).  Three matmul streams per 512-col
    chunk per step: M1 = diag(W2,W2) @ h1; band: s_t = W3.T @ h2 accumulated
    group-major into rows 0-7 of one PSUM tile (4 matmuls with disjoint
    nonzero weight columns); M2 = the original [12,128] W1 on a 12-row fT
    tile (rows 0-9 features h-major, rows 10-11 delta slots).
  * Feedback: one Act-engine copy pst[0:8] -> s_sb per superchunk-step, then
    one SWDGE (gpsimd) DMA into the next step's delta slots and one HWDGE
    DMA to the output row.  Shape-mismatched DMAs ([8,512] -> [2,2048] /
    [1,4096]) exploit element-order run pairing.
  * b3 folding: delta slots carry s_t = W3.T h2 (no b3); the h1 drain bias
    is b1 + b3*w1d; step-0 slots are DMA-initialized to -b3; the host adds
    b3 to the output.
  * 4 lanes (superchunks) run interleaved so the per-step serial chain
    (matmul -> drain -> band -> copy -> DMA -> matmul) hides under the other
    lanes' work.  PSUM drains are relu+bias ops on [128,1024] pairs split
    between the Act and DVE engines.
  * DMA dispatch is scarce (shared HWDGE ~0.63us/DMA serialized; SWDGE holds
    the otherwise-idle GpSimd engine ~1us/DMA): one fT-window load (Act), one
    feedback DMA (gpsimd) and one output DMA (SP) per superchunk-step.
"""

import os

import numpy as np

N, T, FEAT, H = 131072, 60, 5, 64
NCORES = 8
B = N // NCORES            # 16384 paths per core
SC = 4096                  # paths per superchunk (one lane)
NSC = B // SC              # superchunks
G = SC // 2                # paths per group (2 groups stacked on partitions)
CH = 512                   # matmul rhs chunk (fp32 PSUM bank limit)
NCH = G // CH              # chunks per group
LANES = int(os.environ.get("K_LANES", "4"))  # interleaved T-loops
IOBUFS = int(os.environ.get("K_IOBUFS", "6"))    # [128,512] 1-bank io tiles
SBUFS = int(os.environ.get("K_SBUFS", "2"))      # 1-bank band tiles
FWBUFS = int(os.environ.get("K_FWBUFS", str(4 * LANES)))
H1BUFS = int(os.environ.get("K_H1BUFS", str(3 * LANES)))
H2BUFS = int(os.environ.get("K_H2BUFS", str(2 * LANES)))

_BUILD_CACHE = {}


def _build_nc():
    import concourse.bass as bass  # noqa: F401
    import concourse.mybir as mybir
    import concourse.tile as tile
    from concourse import bacc

    f32 = mybir.dt.float32
    bf16 = mybir.dt.bfloat16
    Relu = mybir.ActivationFunctionType.Relu
    add_op = mybir.AluOpType.add
    max_op = mybir.AluOpType.max

    nc = bacc.Bacc("TRN2", target_bir_lowering=False, debug=False)

    featT = nc.dram_tensor("featT", [T * FEAT, B], bf16, kind="ExternalInput")
    wm1_d = nc.dram_tensor("wm1", [128, 128], bf16, kind="ExternalInput")
    w1full_d = nc.dram_tensor("w1full", [12, 128], bf16, kind="ExternalInput")
    w3w_d = nc.dram_tensor("w3w", [128, 8 * NCH], bf16, kind="ExternalInput")
    bias_h2_d = nc.dram_tensor("bias_h2", [128, 1], f32, kind="ExternalInput")
    bias_h1_d = nc.dram_tensor("bias_h1", [128, 1], f32, kind="ExternalInput")
    dinit_d = nc.dram_tensor("dinit", [2, G], bf16, kind="ExternalInput")
    out_d = nc.dram_tensor("deltas", [T, B], bf16, kind="ExternalOutput")

    with tile.TileContext(nc) as tc:
        with (
            tc.tile_pool(name="constp", bufs=1) as constp,
            tc.tile_pool(name="iop", bufs=3) as iop,
            tc.tile_pool(name="statep", bufs=2) as statep,
            tc.tile_pool(name="pspool", bufs=IOBUFS, space="PSUM") as pspool,
        ):
            wm1 = constp.tile_from(wm1_d[:, :], name="wm1_sb")
            w1full = constp.tile_from(w1full_d[:, :], name="w1full_sb")
            w3w = constp.tile_from(w3w_d[:, :], name="w3w_sb")
            bias_h2 = constp.tile_from(bias_h2_d[:, :], name="bias_h2_sb")
            bias_h1 = constp.tile_from(bias_h1_d[:, :], name="bias_h1_sb")

            class Lane:
                pass

            def load_fwin(st, t):
                """Load fT for step t: rows 0-4 group A feats, 5-9 group B
                feats; rows 10/11 are delta slots (DMA-filled at step t-1).
                One shape-mismatched DMA: src [2,5,2048] iterates (h, f, n),
                matching dst partitions 0-9 row-major."""
                fw = iop.tile([12, G], bf16, tag="fTw", bufs=FWBUFS,
                              name="fTw")
                base = st.sc * SC
                src3 = featT[FEAT * t:FEAT * (t + 1), base:base + SC] \
                    .rearrange("f (h n) -> h f n", h=2)
                nc.scalar.dma_start(fw[0:2 * FEAT, :], src3)
                st.fw[t] = fw

            def lane_init(st, sc):
                st.sc = sc
                st.fw = {}
                load_fwin(st, 0)
                load_fwin(st, 1)
                nc.sync.dma_start(st.fw[0][2 * FEAT:2 * FEAT + 2, :],
                                  dinit_d[:, :])
                st.h1 = statep.tile([128, G], bf16, tag="h1", bufs=H1BUFS,
                                    name="h1")
                for c in range(NCH):
                    psl = slice(CH * c, CH * (c + 1))
                    ps = pspool.tile([128, CH], f32, tag="io", name="m2ps")
                    nc.tensor.matmul(ps, w1full, st.fw[0][:, psl],
                                     start=True, stop=True,
                                     skip_group_check=True)
                    if c % 2 == 0:
                        nc.scalar.activation(st.h1[:, psl], ps, Relu,
                                             bias=bias_h1)
                    else:
                        nc.vector.tensor_scalar(st.h1[:, psl], ps, bias_h1,
                                                0.0, add_op, max_op)
                return st

            def phase1(st, t):
                """M1 + drains + band + s-copy."""
                if t + 2 < T:
                    load_fwin(st, t + 2)
                h2 = statep.tile([128, G], bf16, tag="h2", bufs=H2BUFS,
                                 name="h2")
                for c in range(NCH):
                    psl = slice(CH * c, CH * (c + 1))
                    ps = pspool.tile([128, CH], f32, tag="io", name="m1ps")
                    nc.tensor.matmul(ps, wm1, st.h1[:, psl],
                                     start=True, stop=True,
                                     skip_group_check=True)
                    if c < 2 or (c == 2 and t % 2 == 0):
                        nc.scalar.activation(h2[:, psl], ps, Relu,
                                             bias=bias_h2)
                    else:
                        nc.vector.tensor_scalar(h2[:, psl], ps, bias_h2,
                                                0.0, add_op, max_op)
                pst = pspool.tile([128, CH], f32, tag="s", bufs=SBUFS,
                                  name="sband")
                for c in range(NCH):
                    nc.tensor.matmul(
                        pst[0:2 * NCH, :], w3w[:, 8 * c:8 * c + 2 * NCH],
                        h2[:, CH * c:CH * (c + 1)],
                        start=(c == 0), stop=(c == NCH - 1),
                        skip_group_check=True)
                s_sb = iop.tile([2 * NCH, CH], bf16, tag="ssb", bufs=2 * LANES,
                                name="ssb")
                nc.vector.tensor_copy(s_sb[:, :], pst[0:2 * NCH, :])
                st.s_sb = s_sb
                if t - 1 in st.fw:
                    del st.fw[t - 1]

            def sdma(st, t):
                """Feedback + output DMAs for step t's s values.  Emitted a
                few lanes after the s-copy so the queue-head waits on the SP
                and Pool sequencers are already resolved.  s_sb rows are
                group-major, so shape-mismatched DMAs land each group's 4
                chunks contiguously."""
                base = st.sc * SC
                nc.sync.dma_start(out_d[t:t + 1, base:base + SC],
                                  st.s_sb[:, :])
                if t < T - 1:
                    nc.gpsimd.dma_start(
                        st.fw[t + 1][2 * FEAT:2 * FEAT + 2, :], st.s_sb[:, :])

            def phase2(st, t):
                """M2: h1_{t+1} from [fT_{t+1}; s_t] + drains."""
                fw = st.fw[t + 1]
                st.h1 = statep.tile([128, G], bf16, tag="h1", bufs=H1BUFS,
                                    name="h1")
                for c in range(NCH):
                    psl = slice(CH * c, CH * (c + 1))
                    ps = pspool.tile([128, CH], f32, tag="io", name="m2ps")
                    nc.tensor.matmul(ps, w1full, fw[:, psl],
                                     start=True, stop=True,
                                     skip_group_check=True)
                    if c >= 2 or (c == 1 and t % 2 == 1):
                        nc.scalar.activation(st.h1[:, psl], ps, Relu,
                                             bias=bias_h1)
                    else:
                        nc.vector.tensor_scalar(st.h1[:, psl], ps, bias_h1,
                                                0.0, add_op, max_op)

            assert NSC == LANES, "single-pass schedule expects LANES == NSC"
            lanes = [lane_init(Lane(), s) for s in range(LANES)]
            # Rotated software pipeline: each slot emits the PREVIOUS step's
            # M2 first, so every PE instruction in a slot depends only on
            # work from at least one slot earlier (no boundary stall).
            for r in range(T + LANES - 1):
                for i, st in enumerate(lanes):
                    ti = r - 1 - i
                    if 0 <= ti < T - 1:
                        phase2(st, ti)
                for i, st in enumerate(lanes):
                    ti = r - i
                    if 0 <= ti < T:
                        phase1(st, ti)
                for i, st in enumerate(lanes):
                    ti = r - i
                    if 0 <= ti < T:
                        sdma(st, ti)

    nc.compile()
    return nc


def _get_nc():
    if "nc" not in _BUILD_CACHE:
        _BUILD_CACHE["nc"] = _build_nc()
    return _BUILD_CACHE["nc"]


def _host_prep(W1, b1, W2, b2, W3, b3):
    import ml_dtypes
    f32 = np.float32
    bf = ml_dtypes.bfloat16
    W1 = np.asarray(W1, f32)
    b1 = np.asarray(b1, f32)
    W2 = np.asarray(W2, f32)
    b2 = np.asarray(b2, f32)
    W3 = np.asarray(W3, f32)
    b3 = np.asarray(b3, f32)
    W1f = W1[0:FEAT, :]                    # (5, 64)
    w1d = W1[FEAT, :]                      # (64,)

    wm1 = np.zeros((128, 128), f32)
    wm1[0:64, 0:64] = W2
    wm1[64:128, 64:128] = W2

    # fw-tile row layout: 0-4 = group A feats, 5-9 = group B feats,
    # 10 = delta A, 11 = delta B.
    w1full = np.zeros((12, 128), f32)
    w1full[0:FEAT, 0:64] = W1f
    w1full[FEAT:2 * FEAT, 64:128] = W1f
    w1full[2 * FEAT, 0:64] = w1d
    w1full[2 * FEAT + 1, 64:128] = w1d

    # Band weights: chunk c's matmul uses cols [8c, 8c+8); only local cols
    # c (group A) and 4+c (group B) are nonzero, so the 4 accumulating
    # matmuls scatter dot products group-major into PSUM rows 0-7.
    w3w = np.zeros((128, 8 * NCH), f32)
    for c in range(NCH):
        w3w[0:64, 8 * c + c] = W3[:, 0]
        w3w[64:128, 8 * c + NCH + c] = W3[:, 0]

    bias_h2 = np.concatenate([b2, b2]).reshape(128, 1)
    h1b = b1 + b3[0] * w1d
    bias_h1 = np.concatenate([h1b, h1b]).reshape(128, 1)
    dinit = np.full((2, G), -b3[0], f32)

    return dict(wm1=wm1.astype(bf), w1full=w1full.astype(bf),
                w3w=w3w.astype(bf), bias_h2=bias_h2, bias_h1=bias_h1,
                dinit=dinit.astype(bf)), b3[0]


def _run(inputs, trace=False):
    import ml_dtypes
    from concourse.bass_utils import run_bass_kernel_spmd

    bf = ml_dtypes.bfloat16
    features = np.asarray(inputs["features"], np.float32)
    shared, b3v = _host_prep(inputs["W1"], inputs["b1"], inputs["W2"],
                             inputs["b2"], inputs["W3"], inputs["b3"])
    nc = _get_nc()

    in_maps = []
    for i in range(NCORES):
        m = dict(shared)
        # host-side staging: feature-major transpose + bf16 cast
        m["featT"] = np.ascontiguousarray(
            features[i * B:(i + 1) * B].reshape(B, T * FEAT).T).astype(bf)
        in_maps.append(m)

    res = run_bass_kernel_spmd(nc, in_maps, core_ids=list(range(NCORES)),
                               trace=trace)
    # device output is step-major [T, B] bf16 and excludes b3: undo on host
    out = np.concatenate(
        [np.asarray(r["deltas"], np.float32).T for r in res.results], axis=0)
    out += b3v
    return out, res


def kernel(**inputs):
    out, _ = _run(inputs, trace=False)
    return out


def kernel_traced(**inputs):
    return _run(inputs, trace=True)


# revision 28
# speedup vs baseline: 1.0847x; 1.0847x over previous
"""Trainium2 Bass kernel for BaselineFeedforwardNetwork forward_trajectory.

Math (per path, T=60 sequential steps with scalar delta feedback):
    x_t = [f_t (5), d_{t-1}]                       (6,)
    h1  = relu(x_t @ W1 + b1)                      (64,)
    h2  = relu(h1 @ W2 + b2)                       (64,)
    d_t = h2 @ W3 + b3                             scalar
Output: deltas (N, T).

Kernel structure (per core, B = N/8 = 16384 paths, data-parallel over 8 cores):
  * bf16 datapath (weights, activations, staged features, output deltas);
    PSUM accumulation in fp32.  End-to-end error vs the fp32 reference is
    ~8e-3 (the recurrence is contractive).
  * Features are transposed to feature-major [T*FEAT, B] and cast to bf16 on
    the HOST (staging layout choice, like the weight preprocessing); the
    output is written step-major [T, B] and transposed back on the host.
    This removes all on-device transposes: the device runs only the
    recurrence itself.
  * Feature-major activations [hidden, path]; two path groups stacked on 128
    partitions (block-diagonal weights).  Three matmul streams per 512-col
    chunk per step: M1 = diag(W2,W2) @ h1; band: s_t = W3.T @ h2 accumulated
    group-major into rows 0-7 of one PSUM tile (4 matmuls with disjoint
    nonzero weight columns); M2 = the original [12,128] W1 on a 12-row fT
    tile (rows 0-9 features h-major, rows 10-11 delta slots).
  * Feedback: one Act-engine copy pst[0:8] -> s_sb per superchunk-step, then
    one SWDGE (gpsimd) DMA into the next step's delta slots and one HWDGE
    DMA to the output row.  Shape-mismatched DMAs ([8,512] -> [2,2048] /
    [1,4096]) exploit element-order run pairing.
  * b3 folding: delta slots carry s_t = W3.T h2 (no b3); the h1 drain bias
    is b1 + b3*w1d; step-0 slots are DMA-initialized to -b3; the host adds
    b3 to the output.
  * 4 lanes (superchunks) run interleaved so the per-step serial chain
    (matmul -> drain -> band -> copy -> DMA -> matmul) hides under the other
    lanes' work.  PSUM drains are relu+bias ops on [128,1024] pairs split
    between the Act and DVE engines.
  * DMA dispatch is scarce (shared HWDGE ~0.63us/DMA serialized; SWDGE holds
    the otherwise-idle GpSimd engine ~1us/DMA): one fT-window load (Act), one
    feedback DMA (gpsimd) and one output DMA (SP) per superchunk-step.
"""

import os

import numpy as np

N, T, FEAT, H = 131072, 60, 5, 64
NCORES = 8
B = N // NCORES            # 16384 paths per core
SC = 4096                  # paths per superchunk (one lane)
NSC = B // SC              # superchunks
G = SC // 2                # paths per group (2 groups stacked on partitions)
CH = 512                   # matmul rhs chunk (fp32 PSUM bank limit)
NCH = G // CH              # chunks per group
LANES = int(os.environ.get("K_LANES", "4"))  # interleaved T-loops
IOBUFS = int(os.environ.get("K_IOBUFS", "3"))    # [128,1024] 2-bank io tiles
SBUFS = int(os.environ.get("K_SBUFS", "2"))      # 1-bank band tiles
FWBUFS = int(os.environ.get("K_FWBUFS", str(4 * LANES)))
H1BUFS = int(os.environ.get("K_H1BUFS", str(3 * LANES)))
H2BUFS = int(os.environ.get("K_H2BUFS", str(2 * LANES)))

_BUILD_CACHE = {}


def _build_nc():
    import concourse.bass as bass  # noqa: F401
    import concourse.mybir as mybir
    import concourse.tile as tile
    from concourse import bacc

    f32 = mybir.dt.float32
    bf16 = mybir.dt.bfloat16
    Relu = mybir.ActivationFunctionType.Relu
    add_op = mybir.AluOpType.add
    max_op = mybir.AluOpType.max

    nc = bacc.Bacc("TRN2", target_bir_lowering=False, debug=False)

    featT = nc.dram_tensor("featT", [T * FEAT, B], bf16, kind="ExternalInput")
    wm1_d = nc.dram_tensor("wm1", [128, 128], bf16, kind="ExternalInput")
    w1full_d = nc.dram_tensor("w1full", [12, 128], bf16, kind="ExternalInput")
    w3w_d = nc.dram_tensor("w3w", [128, 8 * NCH], bf16, kind="ExternalInput")
    bias_h2_d = nc.dram_tensor("bias_h2", [128, 1], f32, kind="ExternalInput")
    bias_h1_d = nc.dram_tensor("bias_h1", [128, 1], f32, kind="ExternalInput")
    dinit_d = nc.dram_tensor("dinit", [2, G], bf16, kind="ExternalInput")
    out_d = nc.dram_tensor("deltas", [T, B], bf16, kind="ExternalOutput")

    with tile.TileContext(nc) as tc:
        with (
            tc.tile_pool(name="constp", bufs=1) as constp,
            tc.tile_pool(name="iop", bufs=3) as iop,
            tc.tile_pool(name="statep", bufs=2) as statep,
            tc.tile_pool(name="pspool", bufs=IOBUFS, space="PSUM") as pspool,
        ):
            wm1 = constp.tile_from(wm1_d[:, :], name="wm1_sb")
            w1full = constp.tile_from(w1full_d[:, :], name="w1full_sb")
            w3w = constp.tile_from(w3w_d[:, :], name="w3w_sb")
            bias_h2 = constp.tile_from(bias_h2_d[:, :], name="bias_h2_sb")
            bias_h1 = constp.tile_from(bias_h1_d[:, :], name="bias_h1_sb")

            class Lane:
                pass

            def load_fwin(st, t):
                """Load fT for step t: rows 0-4 group A feats, 5-9 group B
                feats; rows 10/11 are delta slots (DMA-filled at step t-1).
                One shape-mismatched DMA: src [2,5,2048] iterates (h, f, n),
                matching dst partitions 0-9 row-major."""
                fw = iop.tile([12, G], bf16, tag="fTw", bufs=FWBUFS,
                              name="fTw")
                base = st.sc * SC
                src3 = featT[FEAT * t:FEAT * (t + 1), base:base + SC] \
                    .rearrange("f (h n) -> h f n", h=2)
                nc.scalar.dma_start(fw[0:2 * FEAT, :], src3)
                st.fw[t] = fw

            def lane_init(st, sc):
                st.sc = sc
                st.fw = {}
                load_fwin(st, 0)
                load_fwin(st, 1)
                nc.sync.dma_start(st.fw[0][2 * FEAT:2 * FEAT + 2, :],
                                  dinit_d[:, :])
                st.h1 = statep.tile([128, G], bf16, tag="h1", bufs=H1BUFS,
                                    name="h1")
                for pair in range(2):
                    psl = slice(2 * CH * pair, 2 * CH * (pair + 1))
                    ps = pspool.tile([128, 2 * CH], f32, tag="io", name="m2ps")
                    for kk in range(2):
                        c = 2 * pair + kk
                        nc.tensor.matmul(
                            ps[:, CH * kk:CH * (kk + 1)], w1full,
                            st.fw[0][:, CH * c:CH * (c + 1)],
                            start=True, stop=True, skip_group_check=True)
                    if pair == 0:
                        nc.scalar.activation(st.h1[:, psl], ps, Relu,
                                             bias=bias_h1)
                    else:
                        nc.vector.tensor_scalar(st.h1[:, psl], ps, bias_h1,
                                                0.0, add_op, max_op)
                return st

            def phase1a(st, t):
                """M1 + drains."""
                h2 = statep.tile([128, G], bf16, tag="h2", bufs=H2BUFS,
                                 name="h2")
                for pair in range(2):
                    psl = slice(2 * CH * pair, 2 * CH * (pair + 1))
                    ps = pspool.tile([128, 2 * CH], f32, tag="io", name="m1ps")
                    for kk in range(2):
                        c = 2 * pair + kk
                        nc.tensor.matmul(
                            ps[:, CH * kk:CH * (kk + 1)], wm1,
                            st.h1[:, CH * c:CH * (c + 1)],
                            start=True, stop=True, skip_group_check=True)
                    if pair == 0 or t % 2 == 0:
                        nc.scalar.activation(h2[:, psl], ps, Relu,
                                             bias=bias_h2)
                    else:
                        nc.vector.tensor_scalar(h2[:, psl], ps, bias_h2,
                                                0.0, add_op, max_op)
                st.h2 = h2

            def phase1b(st, t):
                """Band + s-copy."""
                h2 = st.h2
                pst = pspool.tile([128, CH], f32, tag="s", bufs=SBUFS,
                                  name="sband")
                for c in range(NCH):
                    nc.tensor.matmul(
                        pst[0:2 * NCH, :], w3w[:, 8 * c:8 * c + 2 * NCH],
                        h2[:, CH * c:CH * (c + 1)],
                        start=(c == 0), stop=(c == NCH - 1),
                        skip_group_check=True)
                s_sb = iop.tile([2 * NCH, CH], bf16, tag="ssb", bufs=2 * LANES,
                                name="ssb")
                nc.vector.tensor_copy(s_sb[:, :], pst[0:2 * NCH, :])
                st.s_sb = s_sb
                if t - 1 in st.fw:
                    del st.fw[t - 1]

            def sdma(st, t):
                """Feedback + output DMAs for step t's s values.  Emitted a
                few lanes after the s-copy so the queue-head waits on the SP
                and Pool sequencers are already resolved.  s_sb rows are
                group-major, so shape-mismatched DMAs land each group's 4
                chunks contiguously."""
                base = st.sc * SC
                nc.sync.dma_start(out_d[t:t + 1, base:base + SC],
                                  st.s_sb[:, :])
                if t < T - 1:
                    nc.gpsimd.dma_start(
                        st.fw[t + 1][2 * FEAT:2 * FEAT + 2, :], st.s_sb[:, :])

            def phase2(st, t):
                """M2: h1_{t+1} from [fT_{t+1}; s_t] + drains."""
                fw = st.fw[t + 1]
                st.h1 = statep.tile([128, G], bf16, tag="h1", bufs=H1BUFS,
                                    name="h1")
                for pair in range(2):
                    psl = slice(2 * CH * pair, 2 * CH * (pair + 1))
                    ps = pspool.tile([128, 2 * CH], f32, tag="io", name="m2ps")
                    for kk in range(2):
                        c = 2 * pair + kk
                        nc.tensor.matmul(
                            ps[:, CH * kk:CH * (kk + 1)], w1full,
                            fw[:, CH * c:CH * (c + 1)],
                            start=True, stop=True, skip_group_check=True)
                    if pair == 0:
                        nc.vector.tensor_scalar(st.h1[:, psl], ps, bias_h1,
                                                0.0, add_op, max_op)
                    else:
                        nc.scalar.activation(st.h1[:, psl], ps, Relu,
                                             bias=bias_h1)

            assert NSC == LANES, "single-pass schedule expects LANES == NSC"
            lanes = [lane_init(Lane(), s) for s in range(LANES)]
            # Wave-ordered software pipeline: per slot emit [fwin x4]
            # [prev-step M2 x4] [M1 x4] [band+copy x4] [sdma x4] so each PE
            # instruction has several lanes of ready work queued ahead of
            # the dependency it waits on (no head-of-line blocking).
            for r in range(T + LANES - 1):
                for i, st in enumerate(lanes):
                    ti = r - i
                    if 0 <= ti < T and ti + 2 < T:
                        load_fwin(st, ti + 2)
                for i, st in enumerate(lanes):
                    ti = r - 1 - i
                    if 0 <= ti < T - 1:
                        phase2(st, ti)
                for i, st in enumerate(lanes):
                    ti = r - i
                    if 0 <= ti < T:
                        phase1a(st, ti)
                for i, st in enumerate(lanes):
                    ti = r - i
                    if 0 <= ti < T:
                        phase1b(st, ti)
                for i, st in enumerate(lanes):
                    ti = r - i
                    if 0 <= ti < T:
                        sdma(st, ti)

    nc.compile()
    return nc


def _get_nc():
    if "nc" not in _BUILD_CACHE:
        _BUILD_CACHE["nc"] = _build_nc()
    return _BUILD_CACHE["nc"]


def _host_prep(W1, b1, W2, b2, W3, b3):
    import ml_dtypes
    f32 = np.float32
    bf = ml_dtypes.bfloat16
    W1 = np.asarray(W1, f32)
    b1 = np.asarray(b1, f32)
    W2 = np.asarray(W2, f32)
    b2 = np.asarray(b2, f32)
    W3 = np.asarray(W3, f32)
    b3 = np.asarray(b3, f32)
    W1f = W1[0:FEAT, :]                    # (5, 64)
    w1d = W1[FEAT, :]                      # (64,)

    wm1 = np.zeros((128, 128), f32)
    wm1[0:64, 0:64] = W2
    wm1[64:128, 64:128] = W2

    # fw-tile row layout: 0-4 = group A feats, 5-9 = group B feats,
    # 10 = delta A, 11 = delta B.
    w1full = np.zeros((12, 128), f32)
    w1full[0:FEAT, 0:64] = W1f
    w1full[FEAT:2 * FEAT, 64:128] = W1f
    w1full[2 * FEAT, 0:64] = w1d
    w1full[2 * FEAT + 1, 64:128] = w1d

    # Band weights: chunk c's matmul uses cols [8c, 8c+8); only local cols
    # c (group A) and 4+c (group B) are nonzero, so the 4 accumulating
    # matmuls scatter dot products group-major into PSUM rows 0-7.
    w3w = np.zeros((128, 8 * NCH), f32)
    for c in range(NCH):
        w3w[0:64, 8 * c + c] = W3[:, 0]
        w3w[64:128, 8 * c + NCH + c] = W3[:, 0]

    bias_h2 = np.concatenate([b2, b2]).reshape(128, 1)
    h1b = b1 + b3[0] * w1d
    bias_h1 = np.concatenate([h1b, h1b]).reshape(128, 1)
    dinit = np.full((2, G), -b3[0], f32)

    return dict(wm1=wm1.astype(bf), w1full=w1full.astype(bf),
                w3w=w3w.astype(bf), bias_h2=bias_h2, bias_h1=bias_h1,
                dinit=dinit.astype(bf)), b3[0]


def _run(inputs, trace=False):
    import ml_dtypes
    from concourse.bass_utils import run_bass_kernel_spmd

    bf = ml_dtypes.bfloat16
    features = np.asarray(inputs["features"], np.float32)
    shared, b3v = _host_prep(inputs["W1"], inputs["b1"], inputs["W2"],
                             inputs["b2"], inputs["W3"], inputs["b3"])
    nc = _get_nc()

    in_maps = []
    for i in range(NCORES):
        m = dict(shared)
        # host-side staging: feature-major transpose + bf16 cast
        m["featT"] = np.ascontiguousarray(
            features[i * B:(i + 1) * B].reshape(B, T * FEAT).T).astype(bf)
        in_maps.append(m)

    res = run_bass_kernel_spmd(nc, in_maps, core_ids=list(range(NCORES)),
                               trace=trace)
    # device output is step-major [T, B] bf16 and excludes b3: undo on host
    out = np.concatenate(
        [np.asarray(r["deltas"], np.float32).T for r in res.results], axis=0)
    out += b3v
    return out, res


def kernel(**inputs):
    out, _ = _run(inputs, trace=False)
    return out


def kernel_traced(**inputs):
    return _run(inputs, trace=True)


# revision 29
# speedup vs baseline: 1.1610x; 1.0703x over previous
"""Trainium2 Bass kernel for BaselineFeedforwardNetwork forward_trajectory.

Math (per path, T=60 sequential steps with scalar delta feedback):
    x_t = [f_t (5), d_{t-1}]                       (6,)
    h1  = relu(x_t @ W1 + b1)                      (64,)
    h2  = relu(h1 @ W2 + b2)                       (64,)
    d_t = h2 @ W3 + b3                             scalar
Output: deltas (N, T).

Kernel structure (per core, B = N/8 = 16384 paths, data-parallel over 8 cores):
  * bf16 datapath (weights, activations, staged features, output deltas);
    PSUM accumulation in fp32.  End-to-end error vs the fp32 reference is
    ~8e-3 (the recurrence is contractive).
  * Features are transposed to feature-major [T*FEAT, B] and cast to bf16 on
    the HOST (staging layout choice, like the weight preprocessing); the
    output is written step-major [T, B] and transposed back on the host.
    This removes all on-device transposes: the device runs only the
    recurrence itself.
  * Feature-major activations [hidden, path]; two path groups stacked on 128
    partitions (block-diagonal weights).  Three matmul streams per 512-col
    chunk per step: M1 = diag(W2,W2) @ h1; band: s_t = W3.T @ h2 accumulated
    group-major into rows 0-7 of one PSUM tile (4 matmuls with disjoint
    nonzero weight columns); M2 = the original [12,128] W1 on a 12-row fT
    tile (rows 0-9 features h-major, rows 10-11 delta slots).
  * Feedback: one Act-engine copy pst[0:8] -> s_sb per superchunk-step, then
    one SWDGE (gpsimd) DMA into the next step's delta slots and one HWDGE
    DMA to the output row.  Shape-mismatched DMAs ([8,512] -> [2,2048] /
    [1,4096]) exploit element-order run pairing.
  * b3 folding: delta slots carry s_t = W3.T h2 (no b3); the h1 drain bias
    is b1 + b3*w1d; step-0 slots are DMA-initialized to -b3; the host adds
    b3 to the output.
  * 4 lanes (superchunks) run interleaved so the per-step serial chain
    (matmul -> drain -> band -> copy -> DMA -> matmul) hides under the other
    lanes' work.  PSUM drains are relu+bias ops on [128,1024] pairs split
    between the Act and DVE engines.
  * DMA dispatch is scarce (shared HWDGE ~0.63us/DMA serialized; SWDGE holds
    the otherwise-idle GpSimd engine ~1us/DMA): one fT-window load (Act), one
    feedback DMA (gpsimd) and one output DMA (SP) per superchunk-step.
"""

import os

import numpy as np

N, T, FEAT, H = 131072, 60, 5, 64
NCORES = 8
B = N // NCORES            # 16384 paths per core
SC = 4096                  # paths per superchunk (one lane)
NSC = B // SC              # superchunks
G = SC // 2                # paths per group (2 groups stacked on partitions)
CH = 512                   # matmul rhs chunk (fp32 PSUM bank limit)
NCH = G // CH              # chunks per group
LANES = int(os.environ.get("K_LANES", "4"))  # interleaved T-loops
IOBUFS = int(os.environ.get("K_IOBUFS", "3"))    # [128,1024] 2-bank io tiles
SBUFS = int(os.environ.get("K_SBUFS", "2"))      # 1-bank band tiles
FWBUFS = int(os.environ.get("K_FWBUFS", str(4 * LANES)))
H1BUFS = int(os.environ.get("K_H1BUFS", str(3 * LANES)))
H2BUFS = int(os.environ.get("K_H2BUFS", str(2 * LANES)))

_BUILD_CACHE = {}


def _build_nc():
    import concourse.bass as bass  # noqa: F401
    import concourse.mybir as mybir
    import concourse.tile as tile
    from concourse import bacc

    f32 = mybir.dt.float32
    bf16 = mybir.dt.bfloat16
    Relu = mybir.ActivationFunctionType.Relu
    add_op = mybir.AluOpType.add
    max_op = mybir.AluOpType.max

    nc = bacc.Bacc("TRN2", target_bir_lowering=False, debug=False)

    featT = nc.dram_tensor("featT", [T * FEAT, B], bf16, kind="ExternalInput")
    wm1_d = nc.dram_tensor("wm1", [128, 128], bf16, kind="ExternalInput")
    w1full_d = nc.dram_tensor("w1full", [12, 128], bf16, kind="ExternalInput")
    w3w_d = nc.dram_tensor("w3w", [128, 8 * NCH], bf16, kind="ExternalInput")
    bias_h2_d = nc.dram_tensor("bias_h2", [128, 1], f32, kind="ExternalInput")
    bias_h1_d = nc.dram_tensor("bias_h1", [128, 1], f32, kind="ExternalInput")
    dinit_d = nc.dram_tensor("dinit", [2, G], bf16, kind="ExternalInput")
    out_d = nc.dram_tensor("deltas", [T, B], bf16, kind="ExternalOutput")

    with tile.TileContext(nc) as tc:
        with (
            tc.tile_pool(name="constp", bufs=1) as constp,
            tc.tile_pool(name="iop", bufs=3) as iop,
            tc.tile_pool(name="statep", bufs=2) as statep,
            tc.tile_pool(name="pspool", bufs=IOBUFS, space="PSUM") as pspool,
        ):
            wm1 = constp.tile_from(wm1_d[:, :], name="wm1_sb")
            w1full = constp.tile_from(w1full_d[:, :], name="w1full_sb")
            w3w = constp.tile_from(w3w_d[:, :], name="w3w_sb")
            bias_h2 = constp.tile_from(bias_h2_d[:, :], name="bias_h2_sb")
            bias_h1 = constp.tile_from(bias_h1_d[:, :], name="bias_h1_sb")

            class Lane:
                pass

            def load_fwin(st, t):
                """Load fT for step t: rows 0-4 group A feats, 5-9 group B
                feats; rows 10/11 are delta slots (DMA-filled at step t-1).
                One shape-mismatched DMA: src [2,5,2048] iterates (h, f, n),
                matching dst partitions 0-9 row-major."""
                fw = iop.tile([12, G], bf16, tag="fTw", bufs=FWBUFS,
                              name="fTw")
                base = st.sc * SC
                src3 = featT[FEAT * t:FEAT * (t + 1), base:base + SC] \
                    .rearrange("f (h n) -> h f n", h=2)
                nc.gpsimd.dma_start(fw[0:2 * FEAT, :], src3)
                st.fw[t] = fw

            def lane_init(st, sc):
                st.sc = sc
                st.fw = {}
                load_fwin(st, 0)
                load_fwin(st, 1)
                nc.sync.dma_start(st.fw[0][2 * FEAT:2 * FEAT + 2, :],
                                  dinit_d[:, :])
                st.h1 = statep.tile([128, G], bf16, tag="h1", bufs=H1BUFS,
                                    name="h1")
                for pair in range(2):
                    psl = slice(2 * CH * pair, 2 * CH * (pair + 1))
                    ps = pspool.tile([128, 2 * CH], f32, tag="io", name="m2ps")
                    for kk in range(2):
                        c = 2 * pair + kk
                        nc.tensor.matmul(
                            ps[:, CH * kk:CH * (kk + 1)], w1full,
                            st.fw[0][:, CH * c:CH * (c + 1)],
                            start=True, stop=True, skip_group_check=True)
                    if pair == 0:
                        nc.scalar.activation(st.h1[:, psl], ps, Relu,
                                             bias=bias_h1)
                    else:
                        nc.vector.tensor_scalar(st.h1[:, psl], ps, bias_h1,
                                                0.0, add_op, max_op)
                return st

            def phase1a(st, t):
                """M1 + drains."""
                h2 = statep.tile([128, G], bf16, tag="h2", bufs=H2BUFS,
                                 name="h2")
                for pair in range(2):
                    psl = slice(2 * CH * pair, 2 * CH * (pair + 1))
                    ps = pspool.tile([128, 2 * CH], f32, tag="io", name="m1ps")
                    for kk in range(2):
                        c = 2 * pair + kk
                        nc.tensor.matmul(
                            ps[:, CH * kk:CH * (kk + 1)], wm1,
                            st.h1[:, CH * c:CH * (c + 1)],
                            start=True, stop=True, skip_group_check=True)
                    if pair == 0 or t % 2 == 0:
                        nc.scalar.activation(h2[:, psl], ps, Relu,
                                             bias=bias_h2)
                    else:
                        nc.vector.tensor_scalar(h2[:, psl], ps, bias_h2,
                                                0.0, add_op, max_op)
                st.h2 = h2

            def phase1b(st, t):
                """Band + s-copy."""
                h2 = st.h2
                pst = pspool.tile([128, CH], f32, tag="s", bufs=SBUFS,
                                  name="sband")
                for c in range(NCH):
                    nc.tensor.matmul(
                        pst[0:2 * NCH, :], w3w[:, 8 * c:8 * c + 2 * NCH],
                        h2[:, CH * c:CH * (c + 1)],
                        start=(c == 0), stop=(c == NCH - 1),
                        skip_group_check=True)
                s_sb = iop.tile([2 * NCH, CH], bf16, tag="ssb", bufs=2 * LANES,
                                name="ssb")
                nc.vector.tensor_copy(s_sb[:, :], pst[0:2 * NCH, :])
                st.s_sb = s_sb
                if t - 1 in st.fw:
                    del st.fw[t - 1]

            def sdma(st, t):
                """Feedback + output DMAs for step t's s values.  Emitted a
                few lanes after the s-copy so the queue-head waits on the SP
                and Pool sequencers are already resolved.  s_sb rows are
                group-major, so shape-mismatched DMAs land each group's 4
                chunks contiguously."""
                base = st.sc * SC
                nc.sync.dma_start(out_d[t:t + 1, base:base + SC],
                                  st.s_sb[:, :])
                if t < T - 1:
                    nc.gpsimd.dma_start(
                        st.fw[t + 1][2 * FEAT:2 * FEAT + 2, :], st.s_sb[:, :])

            def phase2(st, t):
                """M2: h1_{t+1} from [fT_{t+1}; s_t] + drains."""
                fw = st.fw[t + 1]
                st.h1 = statep.tile([128, G], bf16, tag="h1", bufs=H1BUFS,
                                    name="h1")
                for pair in range(2):
                    psl = slice(2 * CH * pair, 2 * CH * (pair + 1))
                    ps = pspool.tile([128, 2 * CH], f32, tag="io", name="m2ps")
                    for kk in range(2):
                        c = 2 * pair + kk
                        nc.tensor.matmul(
                            ps[:, CH * kk:CH * (kk + 1)], w1full,
                            fw[:, CH * c:CH * (c + 1)],
                            start=True, stop=True, skip_group_check=True)
                    if pair == 0:
                        nc.vector.tensor_scalar(st.h1[:, psl], ps, bias_h1,
                                                0.0, add_op, max_op)
                    else:
                        nc.scalar.activation(st.h1[:, psl], ps, Relu,
                                             bias=bias_h1)

            assert NSC == LANES, "single-pass schedule expects LANES == NSC"
            lanes = [lane_init(Lane(), s) for s in range(LANES)]
            # Wave-ordered software pipeline: per slot emit [fwin x4]
            # [prev-step M2 x4] [M1 x4] [band+copy x4] [sdma x4] so each PE
            # instruction has several lanes of ready work queued ahead of
            # the dependency it waits on (no head-of-line blocking).
            for r in range(T + LANES - 1):
                for i, st in enumerate(lanes):
                    ti = r - i
                    if 0 <= ti < T and ti + 2 < T:
                        load_fwin(st, ti + 2)
                for i, st in enumerate(lanes):
                    ti = r - 1 - i
                    if 0 <= ti < T - 1:
                        phase2(st, ti)
                for i, st in enumerate(lanes):
                    ti = r - i
                    if 0 <= ti < T:
                        phase1a(st, ti)
                for i, st in enumerate(lanes):
                    ti = r - i
                    if 0 <= ti < T:
                        phase1b(st, ti)
                for i, st in enumerate(lanes):
                    ti = r - i
                    if 0 <= ti < T:
                        sdma(st, ti)

    nc.compile()
    return nc


def _get_nc():
    if "nc" not in _BUILD_CACHE:
        _BUILD_CACHE["nc"] = _build_nc()
    return _BUILD_CACHE["nc"]


def _host_prep(W1, b1, W2, b2, W3, b3):
    import ml_dtypes
    f32 = np.float32
    bf = ml_dtypes.bfloat16
    W1 = np.asarray(W1, f32)
    b1 = np.asarray(b1, f32)
    W2 = np.asarray(W2, f32)
    b2 = np.asarray(b2, f32)
    W3 = np.asarray(W3, f32)
    b3 = np.asarray(b3, f32)
    W1f = W1[0:FEAT, :]                    # (5, 64)
    w1d = W1[FEAT, :]                      # (64,)

    wm1 = np.zeros((128, 128), f32)
    wm1[0:64, 0:64] = W2
    wm1[64:128, 64:128] = W2

    # fw-tile row layout: 0-4 = group A feats, 5-9 = group B feats,
    # 10 = delta A, 11 = delta B.
    w1full = np.zeros((12, 128), f32)
    w1full[0:FEAT, 0:64] = W1f
    w1full[FEAT:2 * FEAT, 64:128] = W1f
    w1full[2 * FEAT, 0:64] = w1d
    w1full[2 * FEAT + 1, 64:128] = w1d

    # Band weights: chunk c's matmul uses cols [8c, 8c+8); only local cols
    # c (group A) and 4+c (group B) are nonzero, so the 4 accumulating
    # matmuls scatter dot products group-major into PSUM rows 0-7.
    w3w = np.zeros((128, 8 * NCH), f32)
    for c in range(NCH):
        w3w[0:64, 8 * c + c] = W3[:, 0]
        w3w[64:128, 8 * c + NCH + c] = W3[:, 0]

    bias_h2 = np.concatenate([b2, b2]).reshape(128, 1)
    h1b = b1 + b3[0] * w1d
    bias_h1 = np.concatenate([h1b, h1b]).reshape(128, 1)
    dinit = np.full((2, G), -b3[0], f32)

    return dict(wm1=wm1.astype(bf), w1full=w1full.astype(bf),
                w3w=w3w.astype(bf), bias_h2=bias_h2, bias_h1=bias_h1,
                dinit=dinit.astype(bf)), b3[0]


def _run(inputs, trace=False):
    import ml_dtypes
    from concourse.bass_utils import run_bass_kernel_spmd

    bf = ml_dtypes.bfloat16
    features = np.asarray(inputs["features"], np.float32)
    shared, b3v = _host_prep(inputs["W1"], inputs["b1"], inputs["W2"],
                             inputs["b2"], inputs["W3"], inputs["b3"])
    nc = _get_nc()

    in_maps = []
    for i in range(NCORES):
        m = dict(shared)
        # host-side staging: feature-major transpose + bf16 cast
        m["featT"] = np.ascontiguousarray(
            features[i * B:(i + 1) * B].reshape(B, T * FEAT).T).astype(bf)
        in_maps.append(m)

    res = run_bass_kernel_spmd(nc, in_maps, core_ids=list(range(NCORES)),
                               trace=trace)
    # device output is step-major [T, B] bf16 and excludes b3: undo on host
    out = np.concatenate(
        [np.asarray(r["deltas"], np.float32).T for r in res.results], axis=0)
    out += b3v
    return out, res


def kernel(**inputs):
    out, _ = _run(inputs, trace=False)
    return out


def kernel_traced(**inputs):
    return _run(inputs, trace=True)
